# revision 1
# baseline (speedup 1.0000x reference)
"""Distributed CLIP-style loss (l2i symmetric CE + g2i NT-Xent) on 8 TRN2 cores.

Strategy: data-parallel row sharding. Each core k receives column-ROTATED
transposed feature matrices (rotation = its global row offset), so the
diagonal (pos-pair) blocks sit at static local column offsets and one SPMD
program serves all 8 cores. Each core computes the LSE rows for its 256
image rows, 256 text rows and 512 z rows; the host sums the per-row partials.

All GEMMs run in bf16 on the PE (fp32 PSUM accumulate). z-normalization is
computed on-device: square -> ones-matmul (partition-broadcast sum(z^2)) ->
reciprocal -> sqrt -> bf16 scale multiply.
"""

import numpy as np
import ml_dtypes

import concourse.bass as bass
import concourse.mybir as mybir
from concourse.tile import TileContext
from concourse.vector_clock import ScopedClock
from concourse import bass_utils


# --- compat patches for the walrus build in this container ---------------
# 1) EVENT_SEMAPHORE_RANGE_CLEAR (InstISA op 176) is rejected ("ISA wrong
#    length"); emit one EventSemaphore sem-wr-imm 0 per semaphore instead.
def _sem_clear_compat(self, sem):
    nums = list(sem) if isinstance(sem, range) else [
        sem.num if hasattr(sem, "num") else int(sem)
    ]
    last = None
    for n in nums:
        last = self.add_instruction(
            mybir.InstEventSemaphore(
                name=self.bass.get_next_instruction_name(),
                ins=[], outs=[],
                sync_info=mybir.SyncInfo(
                    on_wait=[],
                    on_update=[mybir.SyncUpdate(
                        sync_type="semaphore", id=n,
                        update_mode="sem-wr-imm", update_value=0)],
                ),
            )
        )
    return last


bass.BassGpSimd.sem_clear = _sem_clear_compat


# 2) Every instruction in this walrus build has a single sync-wait slot
#    ("Too many sync wait commands" otherwise), while Tile freely attaches
#    several. Post-pass: hoist extra waits onto wait-only EventSemaphore
#    instructions inserted immediately before the instruction on the same
#    engine (sequencers execute in order, so the semantics are identical).
_mw_ctr = [0]


def _split_multi_waits(nc: bass.Bass) -> None:
    for f in nc.m.functions:
        for bb in f.blocks:
            out = []
            changed = False
            for inst in bb.instructions:
                si = inst.sync_info
                waits = list(si.on_wait) if si is not None and si.on_wait else []
                if len(waits) > 1:
                    for w in waits[:-1]:
                        _mw_ctr[0] += 1
                        es = mybir.InstEventSemaphore(
                            name=f"I-mwsplit-{_mw_ctr[0]}",
                            engine=inst.engine,
                            ins=[], outs=[],
                            sync_info=mybir.SyncInfo(on_wait=[w], on_update=[]),
                        )
                        out.append(es)
                    inst.sync_info = mybir.SyncInfo(
                        on_wait=[waits[-1]],
                        on_update=list(si.on_update or []),
                    )
                    changed = True
                out.append(inst)
            if changed:
                bb.instructions = out
# -------------------------------------------------------------------------

B = 2048
D = 1024
NCORES = 8
TEMP = 0.05
INV_TEMP = 1.0 / TEMP
BPC = B // NCORES          # 256 image/text rows per core
ZPC = 2 * B // NCORES      # 512 z rows per core
NCH = D // 128             # 8 contraction chunks
NB_L = B // 512            # 4 psum banks per l2i row-tile
NB_G = 2 * B // 512        # 8 psum banks per g2i row-tile
NT_L = BPC // 128          # 2 l2i row-tiles per core
NT_G = ZPC // 128          # 4 g2i row-tiles per core

BF16 = mybir.dt.bfloat16
F32 = mybir.dt.float32
AF = mybir.ActivationFunctionType

# stats_out column layout ([128, 16] f32 per core)
COL_LSE_IMG = 0   # +t (2)
COL_LSE_TXT = 2   # +t (2)
COL_POS_L2I = 4   # +t (2) raw dot (unscaled)
COL_LSE_G2I = 6   # +t (4)
COL_POS_G2I = 10  # +t (4) raw cosine sim (unscaled)

_cache: dict = {}


def _build_program(ls: float) -> bass.Bass:
    nc = bass.Bass(trn_type="TRN2")
    img_d = nc.dram_tensor("img", [D, B], BF16, kind="ExternalInput")
    txt_d = nc.dram_tensor("txt", [D, B], BF16, kind="ExternalInput")
    z_d = nc.dram_tensor("z", [D, 2 * B], BF16, kind="ExternalInput")
    eye_d = nc.dram_tensor("eye", [128, 128], F32, kind="ExternalInput")
    negeye_d = nc.dram_tensor("negeye", [128, 128], F32, kind="ExternalInput")
    ones_d = nc.dram_tensor("ones", [128, 128], BF16, kind="ExternalInput")
    out_d = nc.dram_tensor("out", [128, 16], F32, kind="ExternalOutput")

    with TileContext(nc) as tc:
        with (
            tc.tile_pool(name="consts", bufs=1) as consts,
            tc.tile_pool(name="feat", bufs=8) as featp,
            tc.tile_pool(name="zstream", bufs=3) as zp,
            tc.tile_pool(name="zsq", bufs=8) as zsqp,
            tc.tile_pool(name="norm", bufs=1) as normp,
            tc.tile_pool(name="stats", bufs=10) as statp,
            tc.tile_pool(name="scratch", bufs=4) as scrp,
            tc.tile_pool(name="mm", bufs=8, space="PSUM") as mmp,
        ):
            eye = consts.tile([128, 128], F32, tag="eye")
            negeye = consts.tile([128, 128], F32, tag="negeye")
            ones = consts.tile([128, 128], BF16, tag="ones")
            nc.sync.dma_start(eye, eye_d[:, :])
            nc.sync.dma_start(negeye, negeye_d[:, :])
            nc.sync.dma_start(ones, ones_d[:, :])

            stats_out = consts.tile([128, 16], F32, tag="statsout")
            nc.vector.memset(stats_out, 0.0)

            # ---- input DMAs: img/txt chunk-interleaved, then z (pass 1) ----
            img_c = []
            txt_c = []
            for c in range(NCH):
                it = featp.tile([128, B], BF16, tag="img")
                tt = featp.tile([128, B], BF16, tag="txt")
                nc.sync.dma_start(it, img_d[c * 128:(c + 1) * 128, :])
                nc.sync.dma_start(tt, txt_d[c * 128:(c + 1) * 128, :])
                img_c.append(it)
                txt_c.append(tt)

            # ================= Phase A: l2i (two sides) =================
            for side in range(2):
                lhs_c = img_c if side == 0 else txt_c
                rhs_c = txt_c if side == 0 else img_c
                ps = [[None] * NB_L for _ in range(NT_L)]
                for t in range(NT_L):
                    for b in range(NB_L):
                        ps[t][b] = mmp.tile([128, 512], F32, tag="ps", name="ps")
                for c in range(NCH):
                    for t in range(NT_L):
                        for b in range(NB_L):
                            nc.tensor.matmul(
                                ps[t][b],
                                lhs_c[c][:, t * 128:(t + 1) * 128],
                                rhs_c[c][:, b * 512:(b + 1) * 512],
                                start=(c == 0),
                                stop=(c == NCH - 1),
                            )
                for t in range(NT_L):
                    maxs = statp.tile([128, NB_L], F32, tag="maxs")
                    sums = statp.tile([128, NB_L], F32, tag="sums")
                    negmax = statp.tile([128, NB_L], F32, tag="negmax")
                    for b in range(NB_L):
                        if side == 0 and b == 0:
                            # raw positive dot: diag of the [128,128] block
                            scr = scrp.tile([128, 128], F32, tag="ttrscr")
                            nc.vector.tensor_mul(
                                scr, ps[t][0][:, t * 128:(t + 1) * 128], eye)
                            nc.vector.reduce_sum(
                                stats_out[:, COL_POS_L2I + t:COL_POS_L2I + t + 1],
                                scr, axis=mybir.AxisListType.X)
                        nc.vector.reduce_max(
                            maxs[:, b:b + 1], ps[t][b], axis=mybir.AxisListType.X
                        )
                        nc.scalar.mul(negmax[:, b:b + 1], maxs[:, b:b + 1], -ls)
                        escr = scrp.tile([128, 512], BF16, tag="escr")
                        nc.scalar.activation(
                            escr, ps[t][b], AF.Exp,
                            bias=negmax[:, b:b + 1], scale=ls,
                            accum_out=sums[:, b:b + 1],
                        )
                    # combine banks: lse = ls*gmax + ln(sum_b S_b * exp(ls*(max_b-gmax)))
                    gmax = statp.tile([128, 1], F32, tag="gmax")
                    nc.vector.reduce_max(gmax, maxs, axis=mybir.AxisListType.X)
                    neggmax = statp.tile([128, 1], F32, tag="neggmax")
                    nc.scalar.mul(neggmax, gmax, -ls)
                    w4 = statp.tile([128, NB_L], F32, tag="w4")
                    nc.scalar.activation(w4, maxs, AF.Exp, bias=neggmax, scale=ls)
                    scr4 = statp.tile([128, NB_L], F32, tag="scr4")
                    S = statp.tile([128, 1], F32, tag="S")
                    nc.vector.tensor_mul(scr4, sums, w4)
                    nc.vector.reduce_sum(S, scr4, axis=mybir.AxisListType.X)
                    lnS = statp.tile([128, 1], F32, tag="lnS")
                    nc.scalar.activation(lnS, S, AF.Ln)
                    gms = statp.tile([128, 1], F32, tag="gms")
                    nc.scalar.mul(gms, gmax, ls)
                    col = (COL_LSE_IMG if side == 0 else COL_LSE_TXT) + t
                    nc.vector.tensor_add(stats_out[:, col:col + 1], lnS, gms)

            # ================= Phase B: z norms =================
            zsq_c = []
            for c in range(NCH):
                zt = zp.tile([128, 2 * B], BF16, tag="zstream")
                nc.sync.dma_start(zt, z_d[c * 128:(c + 1) * 128, :])
                sq = zsqp.tile([128, 2 * B], BF16, tag="zsq")
                nc.scalar.activation(sq, zt, AF.Square)
                zsq_c.append(sq)

            # nsq[j] broadcast to all partitions via ones-matmul, c-outer
            nps = [mmp.tile([128, 512], F32, tag="ps", name="ps") for _ in range(NB_G)]
            for c in range(NCH):
                for b in range(NB_G):
                    nc.tensor.matmul(
                        nps[b], ones, zsq_c[c][:, b * 512:(b + 1) * 512],
                        start=(c == 0), stop=(c == NCH - 1),
                    )
            recip = normp.tile([128, 2 * B], F32, tag="recip")
            invb = normp.tile([128, 2 * B], BF16, tag="invb")
            for b in range(NB_G):
                sl = slice(b * 512, (b + 1) * 512)
                nc.vector.reciprocal(recip[:, sl], nps[b])
                nc.scalar.activation(invb[:, sl], recip[:, sl], AF.Sqrt)

            # pass 2: zn = z * invnorm (bf16, DVE 2x mode)
            zn_c = []
            for c in range(NCH):
                zt = zp.tile([128, 2 * B], BF16, tag="zstream")
                nc.sync.dma_start(zt, z_d[c * 128:(c + 1) * 128, :])
                zn = zsqp.tile([128, 2 * B], BF16, tag="zsq")
                nc.vector.tensor_mul(zn, zt, invb)
                zn_c.append(zn)

            # ================= Phase C: g2i =================
            for t in range(NT_G):
                ps = [mmp.tile([128, 512], F32, tag="ps", name="ps") for _ in range(NB_G)]
                if t == 0:
                    for c in range(NCH):
                        for b in range(NB_G):
                            nc.tensor.matmul(
                                ps[b],
                                zn_c[c][:, t * 128:(t + 1) * 128],
                                zn_c[c][:, b * 512:(b + 1) * 512],
                                start=(c == 0), stop=(c == NCH - 1),
                            )
                else:
                    for b in range(NB_G):
                        for c in range(NCH):
                            nc.tensor.matmul(
                                ps[b],
                                zn_c[c][:, t * 128:(t + 1) * 128],
                                zn_c[c][:, b * 512:(b + 1) * 512],
                                start=(c == 0), stop=(c == NCH - 1),
                            )
                sums8 = statp.tile([128, NB_G], F32, tag="sums8")
                for b in range(NB_G):
                    if b == 0:
                        # mask self-similarity diagonal with -1e30
                        blk = ps[0][:, t * 128:(t + 1) * 128]
                        nc.vector.tensor_add(blk, blk, negeye)
                    if b == NB_G // 2:
                        # positive pair: col (row + 2048) -> bank 4 diag block
                        scr = scrp.tile([128, 128], F32, tag="ttrscr")
                        nc.vector.tensor_mul(
                            scr, ps[b][:, t * 128:(t + 1) * 128], eye)
                        nc.vector.reduce_sum(
                            stats_out[:, COL_POS_G2I + t:COL_POS_G2I + t + 1],
                            scr, axis=mybir.AxisListType.X)
                    escr = scrp.tile([128, 512], BF16, tag="escr")
                    nc.scalar.activation(
                        escr, ps[b], AF.Exp, scale=INV_TEMP,
                        accum_out=sums8[:, b:b + 1],
                    )
                Ssum = statp.tile([128, 1], F32, tag="Ssum")
                nc.vector.reduce_sum(Ssum, sums8, axis=mybir.AxisListType.X)
                nc.scalar.activation(
                    stats_out[:, COL_LSE_G2I + t:COL_LSE_G2I + t + 1], Ssum, AF.Ln
                )

            nc.sync.dma_start(out_d[:, :], stats_out)

    _split_multi_waits(nc)
    return nc


def _get_program(ls: float) -> bass.Bass:
    key = float(ls)
    if key not in _cache:
        _cache[key] = _build_program(key)
    return _cache[key]


def kernel(image_features, gli_features, text_features, logit_scale):
    ls = float(np.asarray(logit_scale))
    nc = _get_program(ls)

    bf = ml_dtypes.bfloat16
    imgT = np.ascontiguousarray(np.asarray(image_features, dtype=np.float32).T)
    txtT = np.ascontiguousarray(np.asarray(text_features, dtype=np.float32).T)
    z = np.concatenate(
        [np.asarray(gli_features, dtype=np.float32),
         np.asarray(image_features, dtype=np.float32)], axis=0)
    zT = np.ascontiguousarray(z.T)

    eye = np.eye(128, dtype=np.float32)
    negeye = (-1e30 * np.eye(128)).astype(np.float32)
    ones = np.ones((128, 128), dtype=bf)

    in_maps = []
    for k in range(NCORES):
        in_maps.append({
            "img": np.ascontiguousarray(np.roll(imgT, -BPC * k, axis=1)).astype(bf),
            "txt": np.ascontiguousarray(np.roll(txtT, -BPC * k, axis=1)).astype(bf),
            "z": np.ascontiguousarray(np.roll(zT, -ZPC * k, axis=1)).astype(bf),
            "eye": eye,
            "negeye": negeye,
            "ones": ones,
        })

    res = bass_utils.run_bass_kernel_spmd(nc, in_maps, core_ids=list(range(NCORES)))
    globals()["LAST_RESULT"] = res
    out = np.stack([r["out"] for r in res.results]).astype(np.float64)  # [8,128,16]

    lse_img = out[:, :, COL_LSE_IMG:COL_LSE_IMG + NT_L].sum()
    lse_txt = out[:, :, COL_LSE_TXT:COL_LSE_TXT + NT_L].sum()
    pos_l2i = out[:, :, COL_POS_L2I:COL_POS_L2I + NT_L].sum()
    l2i = 0.5 * ((lse_img - ls * pos_l2i) / B + (lse_txt - ls * pos_l2i) / B)

    lse_g2i = out[:, :, COL_LSE_G2I:COL_LSE_G2I + NT_G].sum()
    pos_g2i = out[:, :, COL_POS_G2I:COL_POS_G2I + NT_G].sum()
    g2i = (lse_g2i - INV_TEMP * pos_g2i) / (2 * B)

    total = l2i + g2i
    return (np.float32(total), np.float32(l2i), np.float32(g2i))



# revision 14
# speedup vs baseline: 2.4940x; 2.4940x over previous
"""Distributed CLIP-style loss (l2i symmetric CE + g2i NT-Xent) on 8 TRN2 cores.

v2: fp8 DoubleRow matmuls + circulant-banded symmetric g2i.

Each core k owns 256 l2i rows and 512 z rows. Inputs are column-ROTATED
transposed matrices (rotation = the core's global row offset), so one SPMD
program serves all 8 cores and the similarity structure becomes circulant:
the "upper triangle" of the symmetric (4096,4096) g2i sim matrix is, for
row-tile t, the contiguous local column band [128t, 128t+2176). Each pair
{i,j} is computed exactly once (d=(j-i)%4096 in [1,2048]); row-sums get the
missing lower-triangle terms from column sums of the exp matrix, assembled
on the host (d=2048 positive pairs land twice and are subtracted there).

g2i runs on RAW fp8 z (sim = G * rowinv_i * colinv_j applied after the
matmul with one fused scalar_tensor_tensor per psum half), so the sim
matmuls do not wait for the normalization pipeline. Norms come from a
host-shipped fp8 z^2 tensor via a DoubleRow ones-matmul + fast reciprocal.
Column sums of exp use weight-stationary matmuls (E-block as lhsT, ones
column as rhs) accumulating into a single [128,20] psum column so they
come out in partition layout with no cross-bank drains.
"""

import numpy as np
import ml_dtypes

import concourse.bass as bass
import concourse.mybir as mybir
from concourse.tile import TileContext
from concourse import bass_utils


# --- compat patches for the walrus build in this container ---------------
# 1) EVENT_SEMAPHORE_RANGE_CLEAR (InstISA op 176) is rejected ("ISA wrong
#    length"); emit one EventSemaphore sem-wr-imm 0 per semaphore instead.
def _sem_clear_compat(self, sem):
    nums = list(sem) if isinstance(sem, range) else [
        sem.num if hasattr(sem, "num") else int(sem)
    ]
    last = None
    for n in nums:
        last = self.add_instruction(
            mybir.InstEventSemaphore(
                name=self.bass.get_next_instruction_name(),
                ins=[], outs=[],
                sync_info=mybir.SyncInfo(
                    on_wait=[],
                    on_update=[mybir.SyncUpdate(
                        sync_type="semaphore", id=n,
                        update_mode="sem-wr-imm", update_value=0)],
                ),
            )
        )
    return last


bass.BassGpSimd.sem_clear = _sem_clear_compat


# 2) Every instruction in this walrus build has a single sync-wait slot
#    ("Too many sync wait commands" otherwise), while Tile freely attaches
#    several. Post-pass: hoist extra waits onto wait-only EventSemaphore
#    instructions inserted immediately before the instruction on the same
#    engine (sequencers execute in order, so the semantics are identical).
_mw_ctr = [0]


def _split_multi_waits(nc: bass.Bass) -> None:
    for f in nc.m.functions:
        for bb in f.blocks:
            out = []
            changed = False
            for inst in bb.instructions:
                si = inst.sync_info
                waits = list(si.on_wait) if si is not None and si.on_wait else []
                if len(waits) > 1:
                    for w in waits[:-1]:
                        _mw_ctr[0] += 1
                        es = mybir.InstEventSemaphore(
                            name=f"I-mwsplit-{_mw_ctr[0]}",
                            engine=inst.engine,
                            ins=[], outs=[],
                            sync_info=mybir.SyncInfo(on_wait=[w], on_update=[]),
                        )
                        out.append(es)
                    inst.sync_info = mybir.SyncInfo(
                        on_wait=[waits[-1]],
                        on_update=list(si.on_update or []),
                    )
                    changed = True
                out.append(inst)
            if changed:
                bb.instructions = out
# -------------------------------------------------------------------------

B = 2048
D = 1024
N = 2 * B                  # 4096 z rows
NCORES = 8
TEMP = 0.05
INV_TEMP = 1.0 / TEMP
BPC = B // NCORES          # 256 image/text rows per core
ZPC = N // NCORES          # 512 z rows per core
NCP = D // 256             # 4 DoubleRow chunk-pairs
BAND = 2048 + 128          # g2i band width per row-tile
ZCOLS = 128 * 3 + BAND     # 2560 local z columns each core touches
NT_L = BPC // 128          # 2 l2i row-tiles per core
NT_G = ZPC // 128          # 4 g2i row-tiles per core
NBLK = ZCOLS // 128        # 20 column blocks for colacc

F8 = mybir.dt.float8e4
BF16 = mybir.dt.bfloat16
F32 = mybir.dt.float32
AF = mybir.ActivationFunctionType
ALU = mybir.AluOpType
DR = mybir.MatmulPerfMode.DoubleRow

# stats_out column layout ([128, 40] f32 per core)
COL_LMAX = 0    # + side*4 + t*2 + h   (8): per-half row max of raw dots
COL_LSUM = 8    # + side*4 + t*2 + h   (8): per-half sum exp(ls*(x-max))
COL_LPOS = 16   # + t                  (2): raw positive dot (unscaled)
COL_GSUM = 18   # + t*3 + {h0,h1,tail} (12): per-part sum exp(sim/temp)
COL_GPOS = 30   # + t                  (4): raw positive cosine sim

# fallback switches for instructions this walrus build rejects ("ISA wrong
# length"): tensor_tensor_reduce and reciprocal_approx_fast both die there.
USE_TTR = False  # tensor_tensor_reduce (else tensor_mul + reduce_sum)
USE_RAF = False  # reciprocal_approx_fast (else rsqrt = exp(-0.5*ln))
USE_STT = True   # scalar_tensor_tensor (else tensor_scalar + tensor_mul)

_cache: dict = {}


def _diag_extract(nc, workp, src, eye, accum):
    """accum[p] = src[p, p] via eye multiply + row reduce."""
    scr = workp.tile([128, 128], F32, tag="scr", bufs=2, name="scr")
    if USE_TTR:
        nc.vector.tensor_tensor_reduce(
            scr, src, eye, 1.0, 0.0, ALU.mult, ALU.add, accum)
    else:
        nc.vector.tensor_mul(scr, src, eye)
        nc.vector.reduce_sum(accum, scr, axis=mybir.AxisListType.X)


def _scale_rowcol(nc, workp, ps, rowinv, colinv):
    """ps = ps * rowinv (per-partition) * colinv (per-column), in place."""
    if USE_STT:
        nc.vector.scalar_tensor_tensor(
            ps, ps, rowinv, colinv, ALU.mult, ALU.mult)
    else:
        nc.vector.tensor_scalar_mul(ps, ps, rowinv)
        nc.vector.tensor_mul(ps, ps, colinv)


def _build_program(ls: float) -> bass.Bass:
    nc = bass.Bass(trn_type="TRN2")
    img_d = nc.dram_tensor("img", [D, B], F8, kind="ExternalInput")
    txt_d = nc.dram_tensor("txt", [D, B], F8, kind="ExternalInput")
    z_d = nc.dram_tensor("z", [D, ZCOLS], F8, kind="ExternalInput")
    zsq_d = nc.dram_tensor("zsq", [D, ZCOLS], F8, kind="ExternalInput")
    eye_d = nc.dram_tensor("eye", [128, 128], F32, kind="ExternalInput")
    mask0_d = nc.dram_tensor("mask0", [128, 128], F32, kind="ExternalInput")
    maskt_d = nc.dram_tensor("maskt", [128, 128], F32, kind="ExternalInput")
    stats_d = nc.dram_tensor("stats", [128, 40], F32, kind="ExternalOutput")
    colq_d = nc.dram_tensor("colq", [128, 4 * 17], F32, kind="ExternalOutput")

    with TileContext(nc) as tc:
        with (
            tc.tile_pool(name="consts", bufs=1) as consts,
            tc.tile_pool(name="feat", bufs=NCP) as featp,
            tc.tile_pool(name="escr", bufs=3) as escrp,
            tc.tile_pool(name="work", bufs=4) as workp,
            tc.tile_pool(name="mm", bufs=1, space="PSUM") as mmp,
        ):
            eye = consts.tile([128, 128], F32, tag="eye")
            mask0 = consts.tile([128, 128], F32, tag="mask0")
            maskt = consts.tile([128, 128], F32, tag="maskt")
            nc.sync.dma_start(eye, eye_d[:, :])
            nc.sync.dma_start(mask0, mask0_d[:, :])
            nc.sync.dma_start(maskt, maskt_d[:, :])

            ones8 = consts.tile([128, 2, 128], F8, tag="ones8")
            onesb = consts.tile([128, 1], BF16, tag="onesb")
            nc.vector.memset(ones8, 1.0)
            nc.vector.memset(onesb, 1.0)

            stats = consts.tile([128, 40], F32, tag="stats")
            nc.vector.memset(stats, 0.0)

            invn = consts.tile([128, ZCOLS], F32, tag="invn")
            E = consts.tile([128, NT_G, BAND], BF16, tag="E")
            colq_s = consts.tile([128, 4 * 17], F32, tag="colqs")

            # ---- input DMAs: img/txt pair-interleaved, then zsq, then z ----
            img_c = []
            txt_c = []
            for cp in range(NCP):
                it = featp.tile([128, 2, B], F8, tag="img", name="it")
                tt = featp.tile([128, 2, B], F8, tag="txt", name="tt")
                for i in range(2):
                    c = 2 * cp + i
                    nc.sync.dma_start(it[:, i, :], img_d[c * 128:(c + 1) * 128, :])
                    nc.sync.dma_start(tt[:, i, :], txt_d[c * 128:(c + 1) * 128, :])
                img_c.append(it)
                txt_c.append(tt)
            zsq_c = []
            for cp in range(NCP):
                st = featp.tile([128, 2, ZCOLS], F8, tag="zsq", name="st")
                for i in range(2):
                    c = 2 * cp + i
                    nc.sync.dma_start(st[:, i, :], zsq_d[c * 128:(c + 1) * 128, :])
                zsq_c.append(st)
            z_c = []
            for cp in range(NCP):
                zt = featp.tile([128, 2, ZCOLS], F8, tag="z", name="zt")
                for i in range(2):
                    c = 2 * cp + i
                    nc.sync.dma_start(zt[:, i, :], z_d[c * 128:(c + 1) * 128, :])
                z_c.append(zt)

            # ================= Phase A: l2i (two sides) =================
            # Per (side, row-tile, half): [128,1024] psum; per-half max and
            # exp-sum are exported raw and combined into lse on the host.
            for side in range(2):
                lhs_c = img_c if side == 0 else txt_c
                rhs_c = txt_c if side == 0 else img_c
                for t in range(NT_L):
                    for h in range(2):
                        ps = mmp.tile([128, 1024], F32, tag="big", bufs=3,
                                      name="ps")
                        for q in range(2):
                            for cp in range(NCP):
                                nc.tensor.matmul(
                                    ps[:, q * 512:(q + 1) * 512],
                                    lhs_c[cp][:, :, t * 128:(t + 1) * 128],
                                    rhs_c[cp][:, :, h * 1024 + q * 512:
                                              h * 1024 + (q + 1) * 512],
                                    start=(cp == 0), stop=(cp == NCP - 1),
                                    perf_mode=DR,
                                )
                        if side == 0 and h == 0:
                            # raw positive dot: diag of the [128,128] block
                            _diag_extract(
                                nc, workp, ps[:, t * 128:(t + 1) * 128], eye,
                                stats[:, COL_LPOS + t:COL_LPOS + t + 1])
                        col = side * 4 + t * 2 + h
                        mx = stats[:, COL_LMAX + col:COL_LMAX + col + 1]
                        nc.vector.reduce_max(mx, ps, axis=mybir.AxisListType.X)
                        negb = workp.tile([128, 1], F32, tag="negb", name="negb")
                        nc.scalar.mul(negb, mx, -ls)
                        escr = escrp.tile([128, 1024], BF16, tag="escr",
                                          name="escr")
                        nc.scalar.activation(
                            escr, ps, AF.Exp, bias=negb, scale=ls,
                            accum_out=stats[:, COL_LSUM + col:
                                            COL_LSUM + col + 1],
                        )

            # ================= Phase B: z norms =================
            # nsq[j] = sum_d z[d,j]^2 broadcast to all partitions via a
            # DoubleRow ones-matmul over host-shipped zsq; invn = rsqrt.
            nwid = [1024, 1024, 512]
            for i in range(3):
                nb = mmp.tile([128, 1024], F32, tag="big", bufs=3, name="nb")
                for q in range(nwid[i] // 512):
                    off = i * 1024 + q * 512
                    for cp in range(NCP):
                        nc.tensor.matmul(
                            nb[:, q * 512:(q + 1) * 512],
                            ones8,
                            zsq_c[cp][:, :, off:off + 512],
                            start=(cp == 0), stop=(cp == NCP - 1),
                            perf_mode=DR,
                        )
                sl = slice(i * 1024, i * 1024 + nwid[i])
                if USE_RAF:
                    nc.vector.reciprocal_approx_fast(
                        invn[:, sl], nb[:, :nwid[i]])
                    nc.scalar.activation(invn[:, sl], invn[:, sl], AF.Sqrt)
                else:
                    # rsqrt(x) = exp(-0.5*ln(x)) on the accurate ACT tables
                    nc.scalar.activation(invn[:, sl], nb[:, :nwid[i]], AF.Ln)
                    nc.scalar.activation(invn[:, sl], invn[:, sl], AF.Exp,
                                         scale=-0.5)

            # ================= Phase C: g2i (banded symmetric) =================
            for t in range(NT_G):
                # inv-norms of this tile's own rows, in partition layout
                rowinv = workp.tile([128, 1], F32, tag="rinv", bufs=2,
                                    name="rowinv")
                _diag_extract(nc, workp, invn[:, t * 128:(t + 1) * 128],
                              eye, rowinv)
                for h in range(2):
                    ps = mmp.tile([128, 1024], F32, tag="big", bufs=3,
                                  name="ps")
                    for q in range(2):
                        off = t * 128 + h * 1024 + q * 512
                        for cp in range(NCP):
                            nc.tensor.matmul(
                                ps[:, q * 512:(q + 1) * 512],
                                z_c[cp][:, :, t * 128:(t + 1) * 128],
                                z_c[cp][:, :, off:off + 512],
                                start=(cp == 0), stop=(cp == NCP - 1),
                                perf_mode=DR,
                            )
                    # sim = G * rowinv_i * colinv_j (fused, in place)
                    _scale_rowcol(
                        nc, workp, ps, rowinv,
                        invn[:, t * 128 + h * 1024:t * 128 + (h + 1) * 1024])
                    if h == 0:
                        # self-block: keep strict upper (d in [1,127])
                        nc.vector.tensor_add(
                            ps[:, 0:128], ps[:, 0:128], mask0)
                    col = COL_GSUM + t * 3 + h
                    nc.scalar.activation(
                        E[:, t, h * 1024:(h + 1) * 1024], ps, AF.Exp,
                        scale=INV_TEMP,
                        accum_out=stats[:, col:col + 1],
                    )
                # tail block: cols [2048, 2176) of the band, keep d<=2048
                pt = mmp.tile([128, 512], F32, tag="tail", bufs=1, name="pt")
                off = t * 128 + 2048
                for cp in range(NCP):
                    nc.tensor.matmul(
                        pt[:, 0:128],
                        z_c[cp][:, :, t * 128:(t + 1) * 128],
                        z_c[cp][:, :, off:off + 128],
                        start=(cp == 0), stop=(cp == NCP - 1),
                        perf_mode=DR,
                    )
                _scale_rowcol(nc, workp, pt[:, 0:128], rowinv,
                              invn[:, off:off + 128])
                nc.vector.tensor_add(pt[:, 0:128], pt[:, 0:128], maskt)
                # positive pair: diag (d = 2048) of the tail block
                _diag_extract(nc, workp, pt[:, 0:128], eye,
                              stats[:, COL_GPOS + t:COL_GPOS + t + 1])
                col = COL_GSUM + t * 3 + 2
                nc.scalar.activation(
                    E[:, t, 2048:2048 + 128], pt[:, 0:128], AF.Exp,
                    scale=INV_TEMP,
                    accum_out=stats[:, col:col + 1],
                )

                # column sums: weight-stationary matmuls, E block as lhsT,
                # ones column as rhs -> per-(t,block) column in partition
                # layout. Single-shot matmuls (interleaved long-lived psum
                # accumulation groups lose prior contributions on HW); the
                # host sums the 4 tile layers.
                if t == 0:
                    colq = mmp.tile([128, 512], F32, tag="colq", bufs=1,
                                    name="colq")
                for j in range(17):
                    nc.tensor.matmul(
                        colq[:, t * 17 + j:t * 17 + j + 1],
                        E[:, t, j * 128:(j + 1) * 128],
                        onesb,
                    )

            nc.scalar.copy(colq_s, colq[:, 0:4 * 17])
            nc.sync.dma_start(stats_d[:, :], stats)
            nc.sync.dma_start(colq_d[:, :], colq_s)

    _split_multi_waits(nc)
    return nc


def _get_program(ls: float) -> bass.Bass:
    key = float(ls)
    if key not in _cache:
        _cache[key] = _build_program(key)
    return _cache[key]


def kernel(image_features, gli_features, text_features, logit_scale):
    ls = float(np.asarray(logit_scale))
    nc = _get_program(ls)

    f8 = ml_dtypes.float8_e4m3
    imgT = np.ascontiguousarray(np.asarray(image_features, np.float32).T)
    txtT = np.ascontiguousarray(np.asarray(text_features, np.float32).T)
    z = np.concatenate(
        [np.asarray(gli_features, np.float32),
         np.asarray(image_features, np.float32)], axis=0)
    zT = np.ascontiguousarray(z.T)

    eye = np.eye(128, dtype=np.float32)
    r = np.arange(128)
    # mask0: keep strict upper (s > r); maskt: keep s <= r (incl. diag)
    mask0 = np.where(r[None, :] > r[:, None], 0.0, -1e30).astype(np.float32)
    maskt = np.where(r[None, :] <= r[:, None], 0.0, -1e30).astype(np.float32)

    in_maps = []
    for k in range(NCORES):
        zr = np.ascontiguousarray(
            np.roll(zT, -ZPC * k, axis=1)[:, :ZCOLS]).astype(f8)
        zrf = zr.astype(np.float32)
        in_maps.append({
            "img": np.ascontiguousarray(
                np.roll(imgT, -BPC * k, axis=1)).astype(f8),
            "txt": np.ascontiguousarray(
                np.roll(txtT, -BPC * k, axis=1)).astype(f8),
            "z": zr,
            "zsq": (zrf * zrf).astype(f8),
            "eye": eye,
            "mask0": mask0,
            "maskt": maskt,
        })

    res = bass_utils.run_bass_kernel_spmd(nc, in_maps, core_ids=list(range(NCORES)))
    globals()["LAST_RESULT"] = res
    stats = np.stack([r_["stats"] for r_ in res.results]).astype(np.float64)
    colq = np.stack([r_["colq"] for r_ in res.results]).astype(np.float64)

    # ---- l2i: combine per-half (max, sumexp) into lse on the host ----
    lse_sum = np.zeros(2)
    for side in range(2):
        for t in range(NT_L):
            cols = [side * 4 + t * 2 + h for h in range(2)]
            m = stats[:, :, [COL_LMAX + c for c in cols]]      # [8,128,2]
            s = stats[:, :, [COL_LSUM + c for c in cols]]
            M = m.max(axis=2)
            comb = (s * np.exp(ls * (m - M[:, :, None]))).sum(axis=2)
            lse_sum[side] += (ls * M + np.log(comb)).sum()
    pos_l2i = stats[:, :, COL_LPOS:COL_LPOS + NT_L].sum()
    l2i = 0.5 * ((lse_sum[0] - ls * pos_l2i) / B
                 + (lse_sum[1] - ls * pos_l2i) / B)

    # ---- g2i: assemble row sums from row partials + column sums ----
    # per-core row partials rowacc[k, local_row]
    rowacc = np.zeros((NCORES, ZPC))
    pos = np.zeros((NCORES, ZPC))
    for t in range(NT_G):
        sl = slice(t * 128, (t + 1) * 128)
        rowacc[:, sl] = stats[:, :, COL_GSUM + t * 3:COL_GSUM + t * 3 + 3] \
            .sum(axis=2)
        pos[:, sl] = stats[:, :, COL_GPOS + t]
    # column sums: colq[k, i, t*17+j] is local column 128*(t+j) + i
    colsum = np.zeros(N)
    for k in range(NCORES):
        local = np.zeros(ZCOLS)
        for t in range(NT_G):
            lc = colq[k][:, t * 17:(t + 1) * 17]   # [128, 17]
            local[128 * t:128 * t + 2176] += lc.T.reshape(-1)
        gidx = (ZPC * k + np.arange(ZCOLS)) % N
        np.add.at(colsum, gidx, local)
    rows = rowacc.reshape(-1)
    posf = pos.reshape(-1)
    total = rows + colsum - np.exp(INV_TEMP * posf)
    lse = np.log(total)
    g2i = (lse - INV_TEMP * posf).sum() / N

    total_loss = l2i + g2i
    return (np.float32(total_loss), np.float32(l2i), np.float32(g2i))
